# revision 33
# baseline (speedup 1.0000x reference)
"""Multi-head self-attention with RoPE on 8 Trainium2 NeuronCores.

Problem: B=2, S=2048, D_MODEL=2048, 16 heads x d_k=128, causal, RoPE on Q/K.

Sharding (hardcoded): core c -> batch b=c//4, head group g=c%4 (heads 4g..4g+3).
Data parallel on batch, tensor parallel on heads; q/k/v projections column-
sharded, output projection row-sharded with the partial sums reduced on host.

Device kernel, three PE-saturating passes (bf16 operands, f32 PSUM accum):
  pass1: V proj (all 4 heads) + Q/K proj pair0 fused over one sweep of x,
         using all 8 PSUM banks (t0-t3 qe/qo/ke/ko, t4-t7 vacc).  RoPE reads
         drain t0-t3 while the V-matmul tail still runs, so the next q-chunk
         starts with no PE bubble.
  pass2: Q/K proj pair1 (t0-t3) interleaved with pair0 attention (t4-t7) --
         attention has no dependency on the concurrent rope, which hides the
         DVE latency entirely.
  pass3: pair1 attention interleaved with the output projection (shifted one
         q-chunk so attention drains never stall outproj).
  Causal mask: diagonal 128x512 score tiles get a strictly-lower-triangular
  -1e9 added via one extra 128-wide matmul (tri-mask stationary x identity
  moving) accumulated into the same PSUM bank; exp then yields exact zeros.
  Softmax: den via ones-vector matmul, reciprocal_approx_fast (DVE custom op),
  gpsimd partition broadcast.
  RoPE: even/odd d_k pre-permuted into wq/wk rows on host; rotation is 4
  full-width f32 muls + 4 half-width bf16 add/subs per (head-pair, q-chunk).
"""

import sys

sys.path.insert(0, "/opt/trn_rl_repo")

import math

import ml_dtypes
import numpy as np

import concourse.bass as bass
import concourse.mybir as mybir
from concourse import bass_isa
import concourse.tile as tile
from concourse import bacc
from concourse.bass_utils import run_bass_kernel_spmd

f32 = mybir.dt.float32
bf16 = mybir.dt.bfloat16

B = 2
S = 2048
D = 2048
H = 16
DK = 128
H_CORE = 4  # heads per core
DL = H_CORE * DK  # local feature dim 512
ET = D // 128  # 16 e-tiles (contraction over d_model)
QC = S // 512  # 4 q-chunks
THETA = 10000.0
SCALE = 1.0 / math.sqrt(DK)
NEG = -1e9

N_CORES = 8


def _build():
    nc = bacc.Bacc("TRN2", target_bir_lowering=False, debug=False)

    xT_d = nc.dram_tensor("xT", [D, S], bf16, kind="ExternalInput")
    # per-pair flattened qk weights: cols [p*4096 + et*256 + c], c 0:128 even
    # dims (qe stationary), 128:256 odd dims (qo)
    wqf_d = nc.dram_tensor("wqf", [128, 2 * ET * 256], bf16, kind="ExternalInput")
    wkf_d = nc.dram_tensor("wkf", [128, 2 * ET * 256], bf16, kind="ExternalInput")
    # flattened v weights: cols [et*512 + dl]
    wvf_d = nc.dram_tensor("wvf", [128, ET * DL], bf16, kind="ExternalInput")
    woT_d = nc.dram_tensor("woT", [DL, D], bf16, kind="ExternalInput")
    cosT_d = nc.dram_tensor("cosT", [64, S], f32, kind="ExternalInput")
    sinT_d = nc.dram_tensor("sinT", [64, S], f32, kind="ExternalInput")
    mtri_d = nc.dram_tensor("mtri", [128, 128], bf16, kind="ExternalInput")
    ident_d = nc.dram_tensor("ident", [128, 128], bf16, kind="ExternalInput")
    outT_d = nc.dram_tensor("outT", [D, S], bf16, kind="ExternalOutput")

    Exp = mybir.ActivationFunctionType.Exp
    Copy = mybir.ActivationFunctionType.Copy

    with tile.TileContext(nc) as tc:
      with tc.tile_pool(name="const", bufs=1) as const, \
           tc.tile_pool(name="persist", bufs=1) as persist, \
           tc.tile_pool(name="qkp", bufs=1) as qkp, \
           tc.tile_pool(name="wqp", bufs=2) as wqp, \
           tc.tile_pool(name="wkp", bufs=2) as wkp, \
           tc.tile_pool(name="xsp", bufs=16) as xsp, \
           tc.tile_pool(name="ropet", bufs=2) as ropet, \
           tc.tile_pool(name="ptp", bufs=6) as ptp, \
           tc.tile_pool(name="smallp", bufs=2) as smallp, \
           tc.tile_pool(name="stgp", bufs=8) as stgp, \
           tc.tile_pool(name="wop", bufs=1) as wop, \
           tc.tile_pool(name="psum", bufs=1, space="PSUM") as psum:

        V = [persist.tile([128, DL], bf16, tag=f"v{st}", name=f"v{st}")
             for st in range(ET)]
        OT = [persist.tile([DK, S], bf16, tag=f"ot{h}", name=f"ot{h}")
              for h in range(H_CORE)]

        cos2 = const.tile([128, S], f32, tag="cos2", name="cos2")
        sin2 = const.tile([128, S], f32, tag="sin2", name="sin2")
        mtri = const.tile([128, 128], bf16, tag="mtri", name="mtri")
        ident = const.tile([128, 128], bf16, tag="ident", name="ident")
        ones_f = const.tile([128, 1], f32, tag="ones_f", name="ones_f")
        ones = const.tile([128, 1], bf16, tag="ones", name="ones")

        def load_wqk(p, chunks=4, first_split=False):
            wq_all = wqp.tile([128, ET * 256], bf16, tag="wq", name="wq")
            wk_all = wkp.tile([128, ET * 256], bf16, tag="wk", name="wk")
            cw = 4096 // chunks
            segs = []
            for kk in range(chunks):
                if kk == 0 and first_split:
                    segs += [(0, cw // 2), (cw // 2, cw)]
                else:
                    segs.append((kk * cw, (kk + 1) * cw))
            for a, b in segs:
                nc.gpsimd.dma_start(wq_all[:, a:b], wqf_d[:, p * 4096 + a:p * 4096 + b])
                nc.gpsimd.dma_start(wk_all[:, a:b], wkf_d[:, p * 4096 + a:p * 4096 + b])
            return wq_all, wk_all

        def rope(dst0, dst1, ev, od, qs):
            """ev/od: PSUM accumulators (128,512), rows [hA;hB].

            Reads ev fully before od so the PSUM banks free in order for the
            next q-chunk's projection.
            """
            c = cos2[:, qs]
            sn = sin2[:, qs]
            m1 = ropet.tile([128, 512], bf16, tag="m1", name="m1")
            n1 = ropet.tile([128, 512], bf16, tag="n1", name="n1")
            nc.vector.tensor_mul(m1[:], ev[:], c)
            nc.vector.tensor_mul(n1[:], ev[:], sn)
            m2 = ropet.tile([128, 512], bf16, tag="m2", name="m2")
            n2 = ropet.tile([128, 512], bf16, tag="n2", name="n2")
            nc.vector.tensor_mul(m2[:], od[:], sn)
            nc.vector.tensor_mul(n2[:], od[:], c)
            nc.vector.tensor_sub(dst0[0:64, qs], m1[0:64, :], m2[0:64, :])
            nc.vector.tensor_sub(dst1[0:64, qs], m1[64:128, :], m2[64:128, :])
            nc.vector.tensor_add(dst0[64:128, qs], n1[0:64, :], n2[0:64, :])
            nc.vector.tensor_add(dst1[64:128, qs], n1[64:128, :], n2[64:128, :])

        def proj_chunk(qc, wq_all, wk_all, wv_all):
            """QK (+V if wv_all) projection matmuls for one q-chunk.

            Per half-et group: all qe, then qo, ke, ko (then v0..v3) so the
            accumulators finish in rope's read order and the V tail overlaps
            the rope of this chunk.
            """
            qs = slice(qc * 512, (qc + 1) * 512)
            qe = psum.tile([128, 512], f32, tag="t0", name="qe")
            qo = psum.tile([128, 512], f32, tag="t1", name="qo")
            ke = psum.tile([128, 512], f32, tag="t2", name="ko")
            ko = psum.tile([128, 512], f32, tag="t3", name="ko")
            vacc = None
            if wv_all is not None:
                vacc = [psum.tile([128, DL], f32, tag=f"t{4 + i}",
                                  name=f"vacc{i}") for i in range(4)]
            xts = [None] * ET
            for half in range(2):
                ets = range(half * 8, half * 8 + 8)
                for et in ets:
                    xt = xsp.tile([128, 512], bf16, tag="xs", name="xs")
                    rows = slice(et * 128, (et + 1) * 128)
                    if qc == 0 and wv_all is not None and et < 4:
                        # kernel start: halve the first transfers so they land
                        # on two DMA engines in parallel
                        mid = qc * 512 + 256
                        nc.sync.dma_start(xt[:, 0:256], xT_d[rows, qc * 512:mid])
                        nc.sync.dma_start(xt[:, 256:512], xT_d[rows, mid:(qc + 1) * 512])
                    else:
                        nc.sync.dma_start(xt[:], xT_d[rows, qs])
                    xts[et] = xt
                for acc, w_all, coff in (
                    (qe, wq_all, 0), (qo, wq_all, 128),
                    (ke, wk_all, 0), (ko, wk_all, 128),
                ):
                    for et in ets:
                        nc.tensor.matmul(
                            acc[:],
                            w_all[:, et * 256 + coff: et * 256 + coff + 128],
                            xts[et][:],
                            start=(et == 0), stop=(et == ET - 1),
                        )
                if vacc is not None:
                    for sl in range(4):
                        for et in ets:
                            nc.tensor.matmul(
                                vacc[sl][:],
                                xts[et][:, sl * 128:(sl + 1) * 128],
                                wv_all[:, et * DL:(et + 1) * DL],
                                start=(et == 0), stop=(et == ET - 1),
                            )
            return qe, qo, ke, ko, vacc

        def attn(QTp, KTp, p, hi, qc):
            h = 2 * p + hi
            nkt = 4 * qc + 4
            LAG = 2  # scores run LAG tiles ahead of AV so the static PE
            # order hides the exp latency
            qs = slice(qc * 512, (qc + 1) * 512)
            den = psum.tile([1, 512], f32, tag="t6", name="den")
            oacc = psum.tile([128, 512], f32, tag="t7", name="oacc")
            pts = {}

            def consume(kt):
                j = kt - 4 * qc
                o = 128 * j if j > 0 else 0
                cs = slice(o, 512)
                pt = pts.pop(kt)
                nc.tensor.matmul(
                    den[:, cs], ones[:], pt[:, cs],
                    start=(kt == 0), stop=(kt == nkt - 1),
                    skip_group_check=True,
                )
                nc.tensor.matmul(
                    oacc[:, cs], V[kt][:, h * 128:(h + 1) * 128], pt[:, cs],
                    start=(kt == 0), stop=(kt == nkt - 1),
                    skip_group_check=True,
                )

            for kt in range(nkt):
                j = kt - 4 * qc
                o = 128 * j if j > 0 else 0
                cs = slice(o, 512)
                sps = psum.tile(
                    [128, 512], f32, tag=("t4" if kt % 2 == 0 else "t5"),
                    name="sps",
                )
                nc.tensor.matmul(
                    sps[:, cs],
                    KTp[hi][:, kt * 128:(kt + 1) * 128],
                    QTp[hi][:, qc * 512 + o:(qc + 1) * 512],
                    start=True, stop=(j < 0),
                    skip_group_check=True,
                )
                if j >= 0:
                    mc = slice(128 * j, 128 * j + 128)
                    nc.tensor.matmul(
                        sps[:, mc], mtri[:], ident[:],
                        start=False, stop=True,
                        skip_group_check=True,
                    )
                pt = ptp.tile([128, 512], bf16, tag="pt", name="pt")
                nc.scalar.activation(pt[:, cs], sps[:, cs], Exp, scale=SCALE)
                pts[kt] = pt
                if kt >= LAG:
                    consume(kt - LAG)
            for kt in range(max(0, nkt - LAG), nkt):
                consume(kt)
            rec = smallp.tile([1, 512], f32, tag="rec", name="rec")
            nc.vector.reciprocal_approx_fast(rec[:], den[:])
            bc = smallp.tile([128, 512], f32, tag="bc", name="bc")
            nc.gpsimd.partition_broadcast(bc[:], rec[:])
            nc.vector.tensor_mul(OT[h][:, qs], oacc[:], bc[:])

        # ---- pass1: V + QK pair0 over one sweep of x ---------------------
        QT0 = [qkp.tile([DK, S], bf16, tag=f"qt0{i}", name=f"qt0{i}")
               for i in range(2)]
        KT0 = [qkp.tile([DK, S], bf16, tag=f"kt0{i}", name=f"kt0{i}")
               for i in range(2)]
        with tc.tile_pool(name="wvp", bufs=1) as wvp:
            wv_all = wvp.tile([128, ET * DL], bf16, tag="wv", name="wv")
            for kk in range(8):
                cs = slice(kk * 1024, (kk + 1) * 1024)
                nc.scalar.dma_start(wv_all[:, cs], wvf_d[:, cs])
            wq0, wk0 = load_wqk(0, chunks=8, first_split=True)
            # cos/sin tables per q-chunk so rope(0) isn't gated on one big DMA
            for qc in range(QC):
                qs = slice(qc * 512, (qc + 1) * 512)
                nc.gpsimd.dma_start(cos2[0:64, qs], cosT_d[:, qs])
                nc.gpsimd.dma_start(cos2[64:128, qs], cosT_d[:, qs])
                nc.gpsimd.dma_start(sin2[0:64, qs], sinT_d[:, qs])
                nc.gpsimd.dma_start(sin2[64:128, qs], sinT_d[:, qs])
                if qc == 0:
                    nc.gpsimd.dma_start(mtri[:], mtri_d[:, :])
                    nc.gpsimd.dma_start(ident[:], ident_d[:, :])
            nc.vector.memset(ones_f[:], 1.0)
            nc.vector.tensor_copy(ones[:], ones_f[:])
            wq1, wk1 = load_wqk(1)

            for qc in range(QC):
                qs = slice(qc * 512, (qc + 1) * 512)
                qe, qo, ke, ko, vacc = proj_chunk(qc, wq0, wk0, wv_all)
                rope(QT0[0], QT0[1], qe, qo, qs)
                rope(KT0[0], KT0[1], ke, ko, qs)
                for sl in range(4):
                    nc.scalar.activation(V[qc * 4 + sl][:], vacc[sl][:], Copy)

        # ---- pass2: QK pair1 proj interleaved with pair0 attention -------
        QT1 = [qkp.tile([DK, S], bf16, tag=f"qt1{i}", name=f"qt1{i}")
               for i in range(2)]
        KT1 = [qkp.tile([DK, S], bf16, tag=f"kt1{i}", name=f"kt1{i}")
               for i in range(2)]
        wo_h = []
        for hh in range(H_CORE):
            wt = wop.tile([128, D], bf16, tag=f"wo{hh}", name=f"wo{hh}")
            nc.scalar.dma_start(wt[:, 0:1024], woT_d[hh * 128:(hh + 1) * 128, 0:1024])
            nc.scalar.dma_start(wt[:, 1024:2048], woT_d[hh * 128:(hh + 1) * 128, 1024:2048])
            wo_h.append(wt)

        for qc in range(QC):
            qs = slice(qc * 512, (qc + 1) * 512)
            qe, qo, ke, ko, _ = proj_chunk(qc, wq1, wk1, None)
            rope(QT1[0], QT1[1], qe, qo, qs)
            rope(KT1[0], KT1[1], ke, ko, qs)
            attn(QT0, KT0, 0, 0, qc)
            attn(QT0, KT0, 0, 1, qc)

        # ---- pass3: pair1 attention interleaved with output projection ---
        def outproj(qc):
            qs = slice(qc * 512, (qc + 1) * 512)
            for et in range(ET):
                facc = psum.tile([128, 512], f32, tag=f"t{et % 4}",
                                 name="facc")
                for hh in range(H_CORE):
                    nc.tensor.matmul(
                        facc[:],
                        wo_h[hh][:, et * 128:(et + 1) * 128],
                        OT[hh][:, qs],
                        start=(hh == 0), stop=(hh == H_CORE - 1),
                    )
                st = stgp.tile([128, 512], bf16, tag="stg", name="st")
                if qc >= QC - 2:
                    # the last two chunks run after the final attention: keep
                    # their drains off the DVE queue (which still holds the
                    # attention normalize ops)
                    nc.scalar.activation(st[:], facc[:], Copy)
                elif et % 2 == 0:
                    nc.vector.tensor_copy(st[:], facc[:])
                else:
                    nc.scalar.activation(st[:], facc[:], Copy)
                rows = slice(et * 128, (et + 1) * 128)
                if qc == QC - 1 and et >= 12:
                    # halve the very last transfers; second halves issue from
                    # the otherwise-idle gpsimd DGE
                    mid = qc * 512 + 256
                    nc.sync.dma_start(outT_d[rows, qc * 512:mid], st[:, 0:256])
                    nc.gpsimd.dma_start(outT_d[rows, mid:(qc + 1) * 512],
                                        st[:, 256:512])
                else:
                    nc.sync.dma_start(outT_d[rows, qs], st[:])

        for qc in range(QC):
            attn(QT1, KT1, 1, 0, qc)
            attn(QT1, KT1, 1, 1, qc)
            if qc >= 1:
                outproj(qc - 1)
        outproj(QC - 1)

    return nc


_NC = None


def _get_nc():
    global _NC
    if _NC is None:
        _NC = _build()
        _NC.compile()
    return _NC


def _rope_perm_rows():
    """Row permutation applied to wq/wk for one core's 4 heads.

    Per head-pair p: [hA even dims, hB even dims, hA odd dims, hB odd dims]
    so the device sees even/odd deinterleaved, pair-stacked projections.
    Returns indices into the local (4*DK,) head-row block.
    """
    idx = []
    for p in range(2):
        ha, hb = 2 * p, 2 * p + 1
        idx.extend(ha * DK + np.arange(0, DK, 2))
        idx.extend(hb * DK + np.arange(0, DK, 2))
        idx.extend(ha * DK + np.arange(1, DK, 2))
        idx.extend(hb * DK + np.arange(1, DK, 2))
    return np.asarray(idx)


def _host_tables(positions):
    """cos/sin tables (64, S) float32, matching the fp32 reference math."""
    dim_idx = np.arange(0, DK, 2, dtype=np.float32)
    freqs = np.float32(THETA) ** (dim_idx / np.float32(DK))
    angles = positions.astype(np.float32)[:, None] / freqs[None, :]  # (S, 64)
    return (
        np.ascontiguousarray(np.cos(angles).T.astype(np.float32)),
        np.ascontiguousarray(np.sin(angles).T.astype(np.float32)),
    )


def _flat_qk(wT):
    """(D, DL) -> (128, 2*ET*256): cols [p*4096 + et*256 + c]."""
    a = wT.reshape(ET, 128, DL)  # (et, p, dl)
    out = np.empty((128, 2 * ET * 256), dtype=wT.dtype)
    for p in range(2):
        blk = a[:, :, p * 256:(p + 1) * 256]  # (et, 128, 256)
        out[:, p * ET * 256:(p + 1) * ET * 256] = (
            blk.transpose(1, 0, 2).reshape(128, ET * 256)
        )
    return out


def _flat_v(wT):
    """(D, DL) -> (128, ET*DL): cols [et*512 + dl]."""
    a = wT.reshape(ET, 128, DL)
    return np.ascontiguousarray(a.transpose(1, 0, 2).reshape(128, ET * DL))


def _make_in_maps(inputs):
    x = np.asarray(inputs["x"], dtype=np.float32)
    wq = np.asarray(inputs["wq"], dtype=np.float32)
    wk = np.asarray(inputs["wk"], dtype=np.float32)
    wv = np.asarray(inputs["wv"], dtype=np.float32)
    wo = np.asarray(inputs["wo"], dtype=np.float32)
    token_positions = np.asarray(inputs["token_positions"])

    perm = _rope_perm_rows()
    bfc = ml_dtypes.bfloat16

    r = np.arange(128)
    mtri = np.where(r[:, None] < r[None, :], np.float32(NEG), np.float32(0.0))
    ident = np.eye(128, dtype=np.float32)

    in_maps = []
    for c in range(N_CORES):
        b = c // 4
        g = c % 4
        rows = slice(g * DL, (g + 1) * DL)
        cosT, sinT = _host_tables(token_positions[b])
        in_maps.append(
            {
                "xT": np.ascontiguousarray(x[b].T).astype(bfc),
                "wqf": _flat_qk(wq[rows][perm].T).astype(bfc),
                "wkf": _flat_qk(wk[rows][perm].T).astype(bfc),
                "wvf": _flat_v(wv[rows].T).astype(bfc),
                "woT": np.ascontiguousarray(wo[:, rows].T).astype(bfc),
                "cosT": cosT,
                "sinT": sinT,
                "mtri": mtri.astype(bfc),
                "ident": ident.astype(bfc),
            }
        )
    return in_maps


def kernel(x, wq, wk, wv, wo, token_positions):
    nc = _get_nc()
    in_maps = _make_in_maps(
        {
            "x": x,
            "wq": wq,
            "wk": wk,
            "wv": wv,
            "wo": wo,
            "token_positions": token_positions,
        }
    )
    res = run_bass_kernel_spmd(nc, in_maps, list(range(N_CORES)))

    out = np.zeros((B, S, D), dtype=np.float32)
    for c in range(N_CORES):
        out[c // 4] += res.results[c]["outT"].astype(np.float32).T
    return out
